# revision 7
# baseline (speedup 1.0000x reference)
"""Trainium2 Bass kernel for nn_CaptionModel beam-search sampling.

Full inputs -> full outputs. Internally: vocab-sharded tensor parallelism
across 8 NeuronCores. Per step: each core computes logits for its vocab
shard (h @ wo_shard on PE), per-slice top-8 + logsumexp partials (DVE/ACT
reading PSUM directly), a per-core top-10 candidate payload, AllGather of
the 8 payloads, and a redundant global merge + RNN state update on every
core.  wo shard: 5120 columns SBUF-resident, 1163 streamed per step.

Self-contained: hardcodes shapes from the problem spec.
"""
import numpy as np
import concourse.bass as bass
import concourse.bacc as bacc
import concourse.mybir as mybir
import concourse.tile as tile
from concourse.bass_utils import run_bass_kernel_spmd

f32 = mybir.dt.float32
u32 = mybir.dt.uint32
i32 = mybir.dt.int32
A = mybir.AluOpType
AF = mybir.ActivationFunctionType
X = mybir.AxisListType.X

B, D, V, NC = 10, 1024, 50257, 8
KC = D // 128                      # 8 k-chunks
VS = 6283                          # padded shard width; NC*VS = 50264 >= V
RES = 5120                         # resident wo columns (slices 0..9)
NSLICE = 13                        # 10 resident + 2x512 + 139 streamed
LASTSL = VS - 12 * 512             # 139
NSTR = 5                           # streamed tiles of 256 cols
T = 20
LN2 = float(np.log(2.0))
# ln(1+w) on [0,1], degree-12 chebyshev-fit coeffs (f32 horner err ~1.1e-7)
LNC = [6.420764e-11, 1.0, -0.49999943, 0.33332163, -0.24986862, 0.1990866,
       -0.16244107, 0.12920499, -0.093073614, 0.05531036, -0.02431076,
       0.006815031, -0.0008979244]


def build_program(steps=T):
    nc = bacc.Bacc("TRN2", target_bir_lowering=False, debug=False,
                   num_devices=NC)

    wo_in = nc.dram_tensor("wo_in", [128, KC, RES], f32, kind="ExternalInput")
    wos_in = nc.dram_tensor("wos_in", [NSTR, 128, KC, 256], f32,
                            kind="ExternalInput")
    wh_in = nc.dram_tensor("wh_in", [KC, 128, D], f32, kind="ExternalInput")
    ewx_in = nc.dram_tensor("ewx_in", [V, D], f32, kind="ExternalInput")
    h0T_in = nc.dram_tensor("h0T_in", [128, KC, B], f32, kind="ExternalInput")
    lp0_in = nc.dram_tensor("lp0_in", [B, VS], f32, kind="ExternalInput")
    sup_in = nc.dram_tensor("sup_in", [B, LASTSL], f32, kind="ExternalInput")
    unks_in = nc.dram_tensor("unks_in", [B, 1], f32, kind="ExternalInput")
    cbase_in = nc.dram_tensor("cbase_in", [B, 1], f32, kind="ExternalInput")

    o_seq = nc.dram_tensor("o_seq", [B, steps], f32, kind="ExternalOutput")
    o_lp = nc.dram_tensor("o_lp", [B, steps], f32, kind="ExternalOutput")
    o_sum = nc.dram_tensor("o_sum", [B, 1], f32, kind="ExternalOutput")

    with tile.TileContext(nc) as tc:
        with tc.tile_pool(name="cp", bufs=1) as cp, \
             tc.tile_pool(name="wp", bufs=1) as wp, \
             tc.tile_pool(name="dp", bufs=1, space="DRAM") as dp, \
             tc.tile_pool(name="pp", bufs=1, space="PSUM") as pp:

            pay_d = dp.tile([B, 24], f32)

            # ---------- persistent state ----------
            wo = cp.tile([128, KC * RES], f32)          # resident wo columns
            nc.sync.dma_start(wo[:], wo_in[:])
            wo3 = wo[:].rearrange("p (k v) -> p k v", v=RES)
            hT = cp.tile([128, KC * B], f32)
            nc.sync.dma_start(hT[:], h0T_in[:])
            hT3 = hT[:].rearrange("p (k b) -> p k b", b=B)
            BS = cp.tile([B, 2 * steps], f32)           # beam_seq | beam_lp
            nc.vector.memset(BS[:], 0.0)
            bsum = cp.tile([B, 1], f32)
            nc.vector.memset(bsum[:], 0.0)
            g = cp.tile([B, D], f32)                    # h @ wh (also h_row)
            xg = cp.tile([B, D], f32)                   # emb_wx gather / u
            partials = cp.tile([B, 16], f32)
            sup = cp.tile([B, LASTSL], f32)
            nc.sync.dma_start(sup[:], sup_in[:])
            unksel = cp.tile([B, 1], f32)
            nc.sync.dma_start(unksel[:], unks_in[:])
            cbase = cp.tile([B, 1], f32)
            nc.sync.dma_start(cbase[:], cbase_in[:])

            # ---------- constants ----------
            iosc = cp.tile([B, 160], u32)               # iota scratch
            ident10 = cp.tile([B, B], f32)
            nc.gpsimd.iota(iosc.bitcast(i32)[:, 0:B], pattern=[[-1, B]],
                           base=0, channel_multiplier=1)
            nc.vector.tensor_scalar(ident10[:], iosc.bitcast(i32)[:, 0:B],
                                    0, None, op0=A.is_equal)
            icolf = cp.tile([B, 1], f32)
            nc.gpsimd.iota(iosc[:, 0:1], pattern=[[0, 1]], base=0,
                           channel_multiplier=1)
            nc.vector.tensor_copy(icolf[:], iosc[:, 0:1])
            rowmask = cp.tile([B, 1], f32)
            nc.vector.tensor_scalar(rowmask[:], icolf[:], 1.0, -1e10,
                                    op0=A.min, op1=A.mult)
            i104f = cp.tile([B, 104], f32)
            nc.gpsimd.iota(iosc[:, 0:104], pattern=[[1, 104]], base=0,
                           channel_multiplier=0)
            nc.vector.tensor_copy(i104f[:], iosc[:, 0:104])
            i80f = cp.tile([B, 80], f32)
            nc.vector.tensor_copy(i80f[:], iosc[:, 0:80])
            i160f = cp.tile([1, 160], f32)
            nc.gpsimd.iota(iosc[:, 0:160], pattern=[[1, 160]], base=0,
                           channel_multiplier=0)
            nc.vector.tensor_copy(i160f[:], iosc[0:1, 0:160])
            ones11 = cp.tile([1, 1], f32)
            nc.vector.memset(ones11[:], 1.0)
            ones1B = cp.tile([1, B], f32)
            nc.vector.memset(ones1B[:], 1.0)
            neg1000 = cp.tile([B, 1], f32)
            nc.vector.memset(neg1000[:], -1000.0)

            i80v = i80f[:].rearrange("p (s c) -> p s c", c=10)
            i160v = i160f[:].rearrange("p (q r) -> p q r", r=16)

            for t in range(steps):
                # ============ L: logits + per-slice topk + lse ===========
                cv = wp.tile([B, 104], f32, tag="cv", bufs=2)
                ci = wp.tile([B, 104], f32, tag="ci", bufs=2)
                e_unk = wp.tile([B, 1], f32, tag="e_unk", bufs=2)
                for s in range(NSLICE):
                    c0 = s * 512
                    SL = min(512, VS - c0)
                    if t == 0:
                        half = (s % 2) * 512
                        sl = xg[:, half:half + SL]
                        nc.sync.dma_start(sl, lp0_in[:, c0:c0 + SL])
                    else:
                        psl = pp.tile([B, 512], f32, tag="psl", bufs=2)
                        sl = psl[:, :SL]
                        if s < 10:
                            for k in range(KC):
                                nc.tensor.matmul(sl, hT3[:, k, :],
                                                 wo3[:, k, c0:c0 + SL],
                                                 start=(k == 0),
                                                 stop=(k == KC - 1))
                        else:
                            nh = (SL + 255) // 256
                            for hh in range(nh):
                                ti = (s - 10) * 2 + hh
                                CW = min(256, SL - hh * 256)
                                woc = wp.tile([128, KC * 256], f32,
                                              tag="woc", bufs=2)
                                nc.sync.dma_start(woc[:], wos_in[ti])
                                w4 = woc[:].rearrange("p (k c) -> p k c",
                                                      c=256)
                                for k in range(KC):
                                    nc.tensor.matmul(
                                        psl[:, hh * 256:hh * 256 + CW],
                                        hT3[:, k, :], w4[:, k, 0:CW],
                                        start=(k == 0), stop=(k == KC - 1))
                        if s == NSLICE - 1:
                            nc.scalar.activation(e_unk[:],
                                                 sl[:, LASTSL - 8:LASTSL - 7],
                                                 AF.Exp)
                            nc.vector.tensor_tensor(sl, sl, sup[:], op=A.add)
                        junkexp = wp.tile([B, 512], f32, tag="junkexp",
                                          bufs=1)
                        nc.scalar.activation(junkexp[:, :SL], sl, AF.Exp,
                                             accum_out=partials[:, s:s + 1])
                    nc.vector.max(out=cv[:, 8 * s:8 * s + 8], in_=sl)
                    posu = wp.tile([B, 8], u32, tag="posu", bufs=2)
                    nc.vector.max_index(posu[:], cv[:, 8 * s:8 * s + 8], sl)
                    nc.vector.tensor_scalar(ci[:, 8 * s:8 * s + 8], posu[:],
                                            cbase[:], float(c0),
                                            op0=A.add, op1=A.add)

                # ============ G: g = h @ wh ==============================
                if t < steps - 1:
                    for n in range(2):
                        psg = pp.tile([B, D], f32, tag="pg", bufs=2)
                        for k in range(KC):
                            whc = wp.tile([128, 512], f32, tag="whc", bufs=1)
                            nc.sync.dma_start(
                                whc[:], wh_in[k, :, n * 512:(n + 1) * 512])
                            nc.tensor.matmul(psg[:, :512], hT3[:, k, :],
                                             whc[:], start=(k == 0),
                                             stop=(k == KC - 1))
                        nc.vector.tensor_copy(g[:, n * 512:(n + 1) * 512],
                                              psg[:, :512])

                # ============ S: stage-2 + payload =======================
                pay = wp.tile([B, 24], f32, tag="pay", bufs=2)
                scr = wp.tile([B, 104], f32, tag="scr", bufs=1)
                m2s = wp.tile([B, 8], f32, tag="m2s", bufs=1)
                nc.vector.max(out=pay[:, 0:8], in_=cv[:])
                nc.vector.match_replace(out=scr[:], in_to_replace=pay[:, 0:8],
                                        in_values=cv[:], imm_value=-1e30)
                nc.vector.max(out=m2s[:], in_=scr[:])
                nc.vector.tensor_copy(pay[:, 8:10], m2s[:, 0:2])
                p1 = wp.tile([B, 8], u32, tag="p1", bufs=1)
                p2 = wp.tile([B, 8], u32, tag="p2", bufs=1)
                nc.vector.max_index(p1[:], pay[:, 0:8], cv[:])
                nc.vector.max_index(p2[:], m2s[:], scr[:])
                pcat = wp.tile([B, 10], f32, tag="pcat", bufs=1)
                nc.vector.tensor_copy(pcat[:, 0:8], p1[:])
                nc.vector.tensor_copy(pcat[:, 8:10], p2[:, 0:2])
                jkv = wp.tile([B, 104], f32, tag="jkv", bufs=1)
                for r in range(10):
                    eng, jk = nc.vector, jkv
                    eng.scalar_tensor_tensor(
                        out=jk[:], in0=i104f[:], scalar=pcat[:, r:r + 1],
                        in1=ci[:], op0=A.is_equal, op1=A.mult,
                        accum_out=pay[:, 10 + r:11 + r])
                if t == 0:
                    nc.vector.memset(pay[:, 20:21], 0.125)
                else:
                    ssum = wp.tile([B, 1], f32, tag="ssum", bufs=1)
                    nc.vector.reduce_sum(ssum[:], partials[:, 0:NSLICE],
                                         axis=X)
                    nc.vector.scalar_tensor_tensor(
                        out=pay[:, 20:21], in0=e_unk[:], scalar=unksel[:],
                        in1=ssum[:], op0=A.mult, op1=A.add)
                nc.vector.memset(pay[:, 21:24], 0.0)
                gath_d = dp.tile([NC, B, 24], f32, addr_space="Shared",
                                 name=f"gath_{t}", tag=f"gath_{t}")
                nc.sync.dma_start(pay_d[:], pay[:])
                nc.gpsimd.collective_compute(
                    "AllGather", A.bypass,
                    replica_groups=[list(range(NC))],
                    ins=[pay_d[:].opt()], outs=[gath_d[:].opt()])
                gb = wp.tile([B, NC * 24], f32, tag="gb", bufs=1)
                nc.sync.dma_start(gb[:], gath_d[:].rearrange("s b c -> b s c"))
                gb3 = gb[:].rearrange("p (s c) -> p s c", c=24)

                # ============ M: merge (redundant on all cores) ==========
                S = wp.tile([B, 1], f32, tag="S", bufs=1)
                nc.vector.reduce_sum(S[:], gb3[:, :, 20], axis=X)
                Sb = S[:].bitcast(u32)
                e_u = wp.tile([B, 1], u32, tag="e_u", bufs=1)
                nc.vector.tensor_scalar(e_u[:], Sb, 23, None,
                                        op0=A.logical_shift_right)
                m_u = wp.tile([B, 1], u32, tag="m_u", bufs=1)
                nc.vector.tensor_scalar(m_u[:], Sb, 0x7FFFFF, 0x3F800000,
                                        op0=A.bitwise_and, op1=A.bitwise_or)
                wf = wp.tile([B, 1], f32, tag="wf", bufs=1)
                nc.vector.tensor_scalar(wf[:], m_u[:].bitcast(f32), 1.0, None,
                                        op0=A.subtract)
                y = wp.tile([B, 1], f32, tag="y", bufs=1)
                nc.vector.memset(y[:], LNC[-1])
                for ck in LNC[-2::-1]:
                    nc.vector.tensor_scalar(y[:], y[:], wf[:], float(ck),
                                            op0=A.mult, op1=A.add)
                ef = wp.tile([B, 1], f32, tag="ef", bufs=1)
                nc.vector.tensor_copy(ef[:], e_u[:])
                lse = wp.tile([B, 1], f32, tag="lse", bufs=1)
                nc.vector.tensor_scalar(lse[:], ef[:], LN2, -127.0 * LN2,
                                        op0=A.mult, op1=A.add)
                nc.vector.tensor_tensor(lse[:], lse[:], y[:], op=A.add)

                lp80 = wp.tile([B, 80], f32, tag="lp80", bufs=1)
                nc.vector.tensor_scalar(lp80[:], gb3[:, :, 0:10], lse[:],
                                        None, op0=A.subtract)
                sv = wp.tile([B, 80], f32, tag="sv", bufs=1)
                nc.vector.tensor_scalar(sv[:], lp80[:], bsum[:], None,
                                        op0=A.add)
                if t == 0:
                    nc.vector.tensor_scalar(sv[:], sv[:], rowmask[:], None,
                                            op0=A.add)
                FL = wp.tile([B, 48], f32, tag="FL", bufs=1)
                svscr = wp.tile([B, 80], f32, tag="svscr", bufs=1)
                m2f = wp.tile([B, 8], f32, tag="m2f", bufs=1)
                nc.vector.max(out=FL[:, 0:8], in_=sv[:])
                nc.vector.match_replace(out=svscr[:], in_to_replace=FL[:, 0:8],
                                        in_values=sv[:], imm_value=-1e30)
                nc.vector.max(out=m2f[:], in_=svscr[:])
                nc.vector.tensor_copy(FL[:, 8:10], m2f[:, 0:2])
                nc.vector.memset(FL[:, 10:16], -1e30)
                fp1 = wp.tile([B, 8], u32, tag="fp1", bufs=1)
                fp2 = wp.tile([B, 8], u32, tag="fp2", bufs=1)
                nc.vector.max_index(fp1[:], FL[:, 0:8], sv[:])
                nc.vector.max_index(fp2[:], m2f[:], svscr[:])
                fpc = wp.tile([B, 10], f32, tag="fpc", bufs=1)
                nc.vector.tensor_copy(fpc[:, 0:8], fp1[:])
                nc.vector.tensor_copy(fpc[:, 8:10], fp2[:, 0:2])
                nc.vector.tensor_scalar(FL[:, 16:32], FL[:, 0:16], bsum[:],
                                        None, op0=A.subtract)
                jk8v = wp.tile([B, 80], f32, tag="jk8v", bufs=1)
                for r in range(10):
                    eng, jk = nc.vector, jk8v
                    eng.scalar_tensor_tensor(
                        out=jk[:].rearrange("p (s c) -> p s c", c=10),
                        in0=i80v, scalar=fpc[:, r:r + 1],
                        in1=gb3[:, :, 10:20], op0=A.is_equal, op1=A.mult,
                        accum_out=FL[:, 32 + r:33 + r])

                psF = pp.tile([1, 480], f32, tag="psS", bufs=2)
                for j in range(B):
                    nc.tensor.matmul(psF[:, 48 * j:48 * j + 48],
                                     ident10[:, j:j + 1], FL[:],
                                     start=True, stop=True)
                flatSB = wp.tile([1, 480], f32, tag="flatSB", bufs=1)
                nc.vector.tensor_copy(flatSB[:], psF[:])
                fl3 = flatSB[:].rearrange("p (q c) -> p q c", c=48)
                cvt = wp.tile([1, 160], f32, tag="cvt", bufs=1)
                nc.vector.tensor_copy(
                    cvt[:].rearrange("p (q c) -> p q c", c=16),
                    fl3[:, :, 0:16])
                w1 = wp.tile([1, 8], f32, tag="w1", bufs=1)
                w2 = wp.tile([1, 8], f32, tag="w2", bufs=1)
                bscr = wp.tile([1, 160], f32, tag="bscr", bufs=1)
                nc.vector.max(out=w1[:], in_=cvt[:])
                nc.vector.match_replace(
                    out=bscr[:], in_to_replace=w1[:], in_values=cvt[:],
                    imm_value=-1e30)
                nc.vector.max(out=w2[:], in_=bscr[:])
                f1 = wp.tile([1, 8], u32, tag="f1", bufs=1)
                f2 = wp.tile([1, 8], u32, tag="f2", bufs=1)
                nc.vector.max_index(f1[:], w1[:], cvt[:])
                nc.vector.max_index(f2[:], w2[:], bscr[:])
                wcat = wp.tile([1, B], f32, tag="wcat", bufs=1)
                nc.vector.tensor_copy(wcat[:, 0:8], w1[:])
                nc.vector.tensor_copy(wcat[:, 8:10], w2[:, 0:2])
                fcat = wp.tile([1, B], u32, tag="fcat", bufs=1)
                nc.vector.tensor_copy(fcat[:, 0:8], f1[:])
                nc.vector.tensor_copy(fcat[:, 8:10], f2[:, 0:2])
                ff = wp.tile([1, B], f32, tag="ff", bufs=1)
                nc.vector.tensor_copy(ff[:], fcat[:])
                q_u = wp.tile([1, B], u32, tag="q_u", bufs=1)
                nc.vector.tensor_scalar(q_u[:], fcat[:], 4, None,
                                        op0=A.logical_shift_right)
                qf = wp.tile([1, B], f32, tag="qf", bufs=1)
                nc.vector.tensor_copy(qf[:], q_u[:])
                tki = wp.tile([1, B], f32, tag="tki", bufs=1)
                lci = wp.tile([1, B], f32, tag="lci", bufs=1)
                jk6v = wp.tile([1, 160], f32, tag="jk6v", bufs=1)
                for i in range(B):
                    eng, jk = nc.vector, jk6v
                    eng.scalar_tensor_tensor(
                        out=jk[:].rearrange("p (q r) -> p q r", r=16),
                        in0=i160v, scalar=ff[:, i:i + 1],
                        in1=fl3[:, :, 32:48], op0=A.is_equal, op1=A.mult,
                        accum_out=tki[:, i:i + 1])
                    eng.scalar_tensor_tensor(
                        out=jk[:].rearrange("p (q r) -> p q r", r=16),
                        in0=i160v, scalar=ff[:, i:i + 1],
                        in1=fl3[:, :, 16:32], op0=A.is_equal, op1=A.mult,
                        accum_out=lci[:, i:i + 1])

                psQ = pp.tile([B, B], f32, tag="psS", bufs=2)
                nc.tensor.matmul(psQ[:], ones1B[:], qf[:], start=True,
                                 stop=True)
                P = wp.tile([B, B], f32, tag="P", bufs=1)
                nc.vector.tensor_scalar(P[:], psQ[:], icolf[:], None,
                                        op0=A.is_equal)
                psC = pp.tile([B, 1], f32, tag="psS", bufs=2)
                tok_c = wp.tile([B, 1], f32, tag="tok_c", bufs=1)
                nc.tensor.transpose(psC[:], tki[:], ones11[:])
                nc.vector.tensor_copy(tok_c[:], psC[:])
                psC2 = pp.tile([B, 1], f32, tag="psS", bufs=2)
                loc_c = wp.tile([B, 1], f32, tag="loc_c", bufs=1)
                nc.tensor.transpose(psC2[:], lci[:], ones11[:])
                nc.vector.tensor_copy(loc_c[:], psC2[:])
                psC3 = pp.tile([B, 1], f32, tag="psS", bufs=2)
                w_c = wp.tile([B, 1], f32, tag="w_c", bufs=1)
                nc.tensor.transpose(psC3[:], wcat[:], ones11[:])
                nc.vector.tensor_copy(w_c[:], psC3[:])

                # ============ U: state update ============================
                psBS = pp.tile([B, 2 * steps], f32, tag="psS", bufs=2)
                nc.tensor.matmul(psBS[:], P[:], BS[:], start=True, stop=True)
                nc.vector.tensor_copy(BS[:], psBS[:])
                nc.vector.tensor_copy(BS[:, t:t + 1], tok_c[:])
                nc.vector.tensor_copy(BS[:, steps + t:steps + t + 1],
                                      loc_c[:])
                tz = wp.tile([B, 1], u32, tag="tz", bufs=1)
                nc.vector.tensor_scalar(tz[:], tok_c[:], 0.0, None,
                                        op0=A.is_equal)
                nc.vector.select(bsum[:], tz[:], neg1000[:], w_c[:])

                # ============ R: RNN h update ============================
                if t < steps - 1:
                    tok_u = wp.tile([B, 1], u32, tag="tok_u", bufs=1)
                    nc.vector.tensor_copy(tok_u[:], tok_c[:])
                    nc.gpsimd.indirect_dma_start(
                        out=xg[:], out_offset=None, in_=ewx_in[:],
                        in_offset=bass.IndirectOffsetOnAxis(ap=tok_u[:, :1],
                                                            axis=0))
                    psH = pp.tile([B, D], f32, tag="pg", bufs=2)
                    nc.tensor.matmul(psH[:, 0:512], P[:], g[:, 0:512],
                                     start=True, stop=True)
                    nc.tensor.matmul(psH[:, 512:1024], P[:], g[:, 512:1024],
                                     start=True, stop=True)
                    nc.vector.tensor_tensor(xg[:], psH[:], xg[:], op=A.add)
                    nc.scalar.activation(g[:], xg[:], AF.Tanh)
                    for k in range(KC):
                        psT = pp.tile([128, B], f32, tag="psS", bufs=2)
                        nc.tensor.transpose(psT[:],
                                            g[:, 128 * k:128 * (k + 1)],
                                            ident10[:])
                        nc.vector.tensor_copy(hT3[:, k, :], psT[:])

            nc.sync.dma_start(o_seq[:], BS[:, 0:steps])
            nc.sync.dma_start(o_lp[:], BS[:, steps:2 * steps])
            nc.sync.dma_start(o_sum[:], bsum[:])

    nc.compile()
    return nc


def make_inputs(logprobs0, h0, emb, wx, wh, wo):
    """Host-side prep: shards + derived tensors. Returns per-core in_maps."""
    logprobs0 = np.ascontiguousarray(logprobs0, np.float32)
    h0 = np.ascontiguousarray(h0, np.float32)
    emb = np.ascontiguousarray(emb, np.float32)
    wx = np.ascontiguousarray(wx, np.float32)
    wh = np.ascontiguousarray(wh, np.float32)
    wo = np.ascontiguousarray(wo, np.float32)

    ewx = np.ascontiguousarray(emb @ wx)                     # [V, D]
    wh_r = np.ascontiguousarray(wh.reshape(KC, 128, D))      # [k, p, n]
    h0T = np.ascontiguousarray(
        h0.T.reshape(KC, 128, B).transpose(1, 0, 2))         # [p, k, b]

    in_maps = []
    for s in range(NC):
        lo = s * VS
        hi = min(lo + VS, V)
        n = hi - lo
        wo_s = np.zeros((D, VS), np.float32)
        wo_s[:, :n] = wo[:, lo:hi]
        wo_pkv = wo_s.reshape(KC, 128, VS).transpose(1, 0, 2)  # [p, k, v]
        wo_res = np.ascontiguousarray(wo_pkv[:, :, :RES])
        wo_str = np.zeros((NSTR, 128, KC, 256), np.float32)
        for ti in range(NSTR):
            a = RES + ti * 256
            b = min(a + 256, VS)
            wo_str[ti, :, :, : b - a] = wo_pkv[:, :, a:b].transpose(0, 1, 2)
        lp0 = np.full((B, VS), -1e30, np.float32)
        lp0[:, :n] = logprobs0[:, lo:hi]
        supb = np.zeros((B, LASTSL), np.float32)
        supb[:, n - 12 * 512:] = -1e30                        # pad mask
        unk = 0.0
        if lo <= V - 1 < hi:
            lp0[:, V - 1 - lo] += -1000.0
            supb[:, V - 1 - lo - 12 * 512] += -1000.0
            unk = 1.0
            assert V - 1 - lo - 12 * 512 == LASTSL - 8
        in_maps.append(dict(
            wo_in=wo_res,
            wos_in=np.ascontiguousarray(wo_str),
            wh_in=wh_r,
            ewx_in=ewx,
            h0T_in=h0T,
            lp0_in=np.ascontiguousarray(lp0),
            sup_in=np.ascontiguousarray(supb),
            unks_in=np.full((B, 1), unk, np.float32),
            cbase_in=np.full((B, 1), float(lo), np.float32),
        ))
    return in_maps


_prog_cache = {}


def kernel(logprobs0, h0, emb, wx, wh, wo, seq_length):
    steps = int(seq_length)
    if steps not in _prog_cache:
        _prog_cache[steps] = build_program(steps)
    nc = _prog_cache[steps]
    in_maps = make_inputs(logprobs0, h0, emb, wx, wh, wo)
    res = run_bass_kernel_spmd(nc, in_maps, core_ids=list(range(NC)))
    r0 = res.results[0]
    beam_seq = np.rint(r0["o_seq"].T).astype(np.int32)       # [T, B]
    beam_lp = np.ascontiguousarray(r0["o_lp"].T, np.float32)  # [T, B]
    beam_sum = np.ascontiguousarray(r0["o_sum"][:, 0], np.float32)  # [B]
    return beam_seq, beam_lp, beam_sum


if __name__ == "__main__":
    import reference
    inputs = reference.setup_inputs()
    out = kernel(**{k: np.asarray(v) if hasattr(v, "shape") else v
                    for k, v in inputs.items()})
    print("seq:\n", out[0])
    print("sum:", out[2])
